# revision 2
# baseline (speedup 1.0000x reference)
"""Holt-Winters exponential smoothing (level/trend/seasonal, P=7) on 8 Trainium2
NeuronCores — v12: v11 restructured around the trace findings.

Trace of v11 (152.7 us) showed: PE cold (HAM 4/8) nearly everywhere, g0 scan
stretched 18->41.5 us by input-DMA sem waits (xg0 shared one SWDGE ring with
xg2), pass-2 start 48.9 us, stores trailing chunk production.
v12 changes:
  - PE warm-up: 48 N=128 dummy matmuls on a memset scratch tile while inputs
    load, so the scan runs at 2.4 GHz;
  - xg0 pieces alternate sync/gpsimd rings (both rings pull g0 first);
  - scan h0/h1 chains interleaved pairwise (shared lhsT, LDW amortized);
  - scans for g1/g2 issued in 4 segments between pass-2 chunks i=3..6 so PE
    never idles long enough to re-throttle and chunk production never stops;
  - pass-2 PSUM retiled to 3 x (105,1024) fp32 two-bank tiles, one per
    channel; casts are (105,1024) wide (halves per-instruction overhead),
    alternating DVE/ACT; ot layout ch-major [c0h0,c0h1,c1h0,...];
  - stores alternate sync/gpsimd by chunk parity; sigma scatters ride the
    otherwise-idle scalar (HWDGE) ring; xg1 loads on scalar ring after the
    g0 scatters.
Math identical to v11 (C=105 chunks, 13/group, 3 groups, bf16 weights/x/out,
fp32 PSUM, sigma_hi-only pass-2, bf16-hi group state; relL2 ~4.5e-3).
"""

import numpy as np

P = 7
C = 105
G = 13
NG = 3
NCH = G * NG
KS = 114          # pass-2 rhs rows: 105 X + 9 sigma_hi
L = 4096
B = 8192
NCORES = 8
BL = B // NCORES
NH = 512

# wall (weights+s0) column offsets
WP0 = 0           # (114, 315)
WQ0 = 315         # (105, 1638)
WS0 = WP0 + 315 + 1638          # ws1 (9, 126)
S00 = WS0 + 126                 # s0 (9, 2048)
WALLW = S00 + 2 * BL            # 4127

NDUM = 48         # PE warm-up matmuls (N=128, ~4.3us cold -> HAM 8/8)


def _sigmoid(z):
    return 1.0 / (1.0 + np.exp(-z))


def _step_mats(a, b, g):
    A, c = [], []
    for i in range(P):
        col = 2 + i
        Ai = np.zeros((9, 9), np.float64)
        ci = np.zeros(9, np.float64)
        Ai[0, 0] = 1 - a
        Ai[0, 1] = 1 - a
        Ai[0, col] += -a
        Ai[1, 0] = -a * b
        Ai[1, 1] = 1 - a * b
        Ai[1, col] += -a * b
        for j in range(P):
            Ai[2 + j, 2 + j] = 1.0
        Ai[col, :] = 0.0
        Ai[col, 0] = -g * (1 - a)
        Ai[col, 1] = -g * (1 - a)
        Ai[col, col] = g * a + 1 - g
        ci[0] = a
        ci[1] = a * b
        ci[col] = g * (1 - a)
        A.append(Ai)
        c.append(ci)
    return A, c


def _build_coeffs(alpha, beta, gamma):
    """Weight blocks in float64; packed into the per-core wall later."""
    a, b, g = _sigmoid(alpha), _sigmoid(beta), _sigmoid(gamma)
    A, c = _step_mats(a, b, g)
    slots = [(1 + k) % P for k in range(C)]

    Phi = np.zeros((C, 9, 9), np.float64)
    w = np.zeros((C, C, 9), np.float64)
    cur = np.eye(9)
    for k in range(C):
        i = slots[k]
        if k > 0:
            w[k, :k] = w[k - 1, :k] @ A[i].T
        w[k, k] = c[i]
        cur = A[i] @ cur
        Phi[k] = cur
    T = Phi[C - 1]
    V = w[C - 1].T.copy()

    wp = np.zeros((KS, 3 * C), np.float64)          # [ch0|ch1|ch2]
    for k in range(C):
        sel = [0, 1, 2 + slots[k]]
        for ch in range(3):
            wp[105:114, ch * C + k] = Phi[k][sel[ch]]
            for j in range(k + 1):
                wp[j, ch * C + k] = w[k, j][sel[ch]]

    Tpow = [np.eye(9)]
    for _ in range(G + 1):
        Tpow.append(T @ Tpow[-1])

    ws1 = np.zeros((9, 126), np.float64)
    ws1[:, 0:9] = Tpow[G].T
    for j in range(G):
        ws1[:, 9 + 9 * j:18 + 9 * j] = Tpow[j].T
    wq = np.zeros((C, G * 126), np.float64)         # [i0|i1|...|i12]
    for i in range(G):
        blk = wq[:, i * 126:(i + 1) * 126]
        blk[:, 0:9] = (Tpow[G - 1 - i] @ V).T
        for j in range(i + 1, G):
            blk[:, 9 + 9 * j:18 + 9 * j] = (Tpow[j - 1 - i] @ V).T

    return wp, wq, ws1


def build_bass(bl=BL):
    import concourse.bacc as bacc
    import concourse.mybir as mybir
    from concourse.tile import TileContext

    BF = mybir.dt.bfloat16
    F32 = mybir.dt.float32
    COPY = mybir.ActivationFunctionType.Copy
    GW = G * bl

    nc = bacc.Bacc(None, target_bir_lowering=False, debug=False)
    xin = nc.declare_dram_parameter("xin", [C, NCH * bl], BF, isOutput=False)
    wall_d = nc.declare_dram_parameter("wall", [KS, WALLW], BF,
                                       isOutput=False)
    out_d = nc.declare_dram_parameter("out", [C, NCH * 3 * bl], BF,
                                      isOutput=True)

    with TileContext(nc) as tc:
        with (
            tc.tile_pool(name="consts", bufs=1) as consts,
            tc.tile_pool(name="xpool", bufs=NG) as xpool,
            tc.tile_pool(name="spool", bufs=2) as spool,
            tc.tile_pool(name="ypool", bufs=8) as ypool,
            tc.tile_pool(name="ypsum", bufs=3, space="PSUM") as ypsum,
            tc.tile_pool(name="spsum", bufs=2, space="PSUM") as spsum,
        ):
            cw = consts.tile([KS, WALLW], BF)
            # s0 block first (tiny, gates the first scan matmuls), then the
            # weight block; rows 9:114 of the s0 region are never read.
            nc.sync.dma_start(out=cw[0:9, S00:WALLW], in_=wall_d[0:9, S00:WALLW])
            nc.sync.dma_start(out=cw[:, 0:S00], in_=wall_d[:, 0:S00])
            wp = cw[:, WP0:WP0 + 3 * C]
            wq = cw[0:C, WQ0:WQ0 + G * 126]
            ws1 = cw[0:9, WS0:WS0 + 126]
            s0 = cw[0:9, S00:S00 + 2 * bl]

            # PE warm-up scratch (memset on gpsimd, runs in the preamble)
            dum = consts.tile([128, 128], BF)
            nc.gpsimd.memset(dum[:], 0.0)

            # xg0 in 2-chunk pieces alternating sync/gpsimd so both rings
            # pull group 0 first and the scan is never input-starved.
            xg = []
            xt0 = xpool.tile([KS, GW], BF, tag="xg", name="xg0")
            for j, (a, b) in enumerate([(0, 2), (2, 4), (4, 6), (6, 8),
                                        (8, 10), (10, 12), (12, 13)]):
                src = xin[:, a * bl:b * bl]
                dst = xt0[0:C, a * bl:b * bl]
                if j % 2 == 0:
                    nc.sync.dma_start(out=dst, in_=src)
                else:
                    nc.gpsimd.dma_start(out=dst, in_=src)
            xg.append(xt0)

            # PE warm-up: back-to-back N=128 matmuls into a scratch PSUM
            # tile (spsum buf 0, recycled before the scan needs it). ~4.3us
            # of continuous PE busy flips HAM to 8/8 before the scan.
            dps = spsum.tile([126, NH], F32, tag="sp", name="dps")
            for _ in range(NDUM):
                nc.tensor.matmul(dps[0:126, 0:128], lhsT=dum[:, 0:126],
                                 rhs=dum[:, 0:128], start=True, stop=True)

            # xg1 on the scalar (HWDGE) ring -- issued here, but the g0
            # sigma scatters are queued on scalar first (inside scan(0)
            # below)... scalar ring order follows program order, so load
            # xg1 after scan(0) is issued. xg2 on gpsimd behind xg0 odds.
            xt1 = xpool.tile([KS, GW], BF, tag="xg", name="xg1")
            xg.append(xt1)
            xt2 = xpool.tile([KS, GW], BF, tag="xg", name="xg2")
            for (a, b) in [(0, 7), (7, 13)]:
                nc.gpsimd.dma_start(out=xt2[0:C, a * bl:b * bl],
                                    in_=xin[:, 2 * GW + a * bl:2 * GW + b * bl])
            xg.append(xt2)

            state = [s0[:, 0:bl]]
            sg_tiles = []

            def scan_mm_pairs(g_):
                """The 14 (lhsT, rhs-col) matmul pairs of group g_'s scan,
                h0/h1 chains interleaved so each lhsT is used twice in a row
                (LDW amortized by the PE reorder window)."""
                st = state[g_]
                sp0 = spsum.tile([126, NH], F32, tag="sp", name=f"sp{g_}_0")
                sp1 = spsum.tile([126, NH], F32, tag="sp", name=f"sp{g_}_1")
                pairs = []

                def emit(k):
                    if k == 0:
                        lh, r0, r1 = ws1, st[:, 0:NH], st[:, NH:2 * NH]
                        nc.tensor.matmul(sp0[:], lhsT=lh, rhs=r0,
                                         start=True, stop=False)
                        nc.tensor.matmul(sp1[:], lhsT=lh, rhs=r1,
                                         start=True, stop=False)
                    else:
                        i = k - 1
                        lh = wq[:, i * 126:(i + 1) * 126]
                        base = i * bl
                        last = (i == G - 1)
                        nc.tensor.matmul(
                            sp0[:], lhsT=lh,
                            rhs=xg[g_][0:C, base:base + NH],
                            start=False, stop=last)
                        nc.tensor.matmul(
                            sp1[:], lhsT=lh,
                            rhs=xg[g_][0:C, base + NH:base + 2 * NH],
                            start=False, stop=last)
                return sp0, sp1, emit

            def scan_finish(g_, sp0, sp1):
                """Casts + sigma scatter after the chains complete."""
                sg = spool.tile([126, bl], BF, tag="sg", name=f"sg{g_}")
                nc.scalar.activation(out=sg[:, 0:NH], in_=sp0[:], func=COPY)
                nc.scalar.activation(out=sg[:, NH:2 * NH], in_=sp1[:],
                                     func=COPY)
                for i in range(G):
                    nc.scalar.dma_start(
                        out=xg[g_][105:114, i * bl:(i + 1) * bl],
                        in_=sg[9 + 9 * i:18 + 9 * i, :])
                state.append(sg[0:9, :])
                sg_tiles.append(sg)

            def pass2_chunk(g_, i):
                k = g_ * G + i
                ot = ypool.tile([C, 3 * bl], BF, tag="ot", name=f"ot{k}")
                for ch in range(3):
                    yp = ypsum.tile([C, bl], F32, tag="yp",
                                    name=f"yp{k}_{ch}")
                    for h in range(2):
                        nc.tensor.matmul(
                            yp[:, h * NH:(h + 1) * NH],
                            lhsT=wp[:, ch * C:(ch + 1) * C],
                            rhs=xg[g_][0:KS, i * bl + h * NH:
                                       i * bl + (h + 1) * NH],
                            start=True, stop=True)
                    oc = slice(ch * bl, (ch + 1) * bl)
                    if (k * 3 + ch) % 2 == 0:
                        nc.vector.tensor_copy(out=ot[:, oc], in_=yp[:])
                    else:
                        nc.scalar.activation(out=ot[:, oc], in_=yp[:],
                                             func=COPY)
                c0 = k * 3 * bl
                if k % 2 == 0:
                    nc.sync.dma_start(out=out_d[:, c0:c0 + 3 * bl], in_=ot[:])
                else:
                    nc.gpsimd.dma_start(out=out_d[:, c0:c0 + 3 * bl],
                                        in_=ot[:])

            # ---- group 0 scan (monolithic: competes only with dummies) ----
            sp0, sp1, emit = scan_mm_pairs(0)
            for kk in range(1 + G):
                emit(kk)
            scan_finish(0, sp0, sp1)
            # xg1 load on scalar ring, behind the g0 scatters
            for (a, b) in [(0, 7), (7, 13)]:
                nc.scalar.dma_start(out=xt1[0:C, a * bl:b * bl],
                                    in_=xin[:, GW + a * bl:GW + b * bl])

            # ---- pass-2 with segmented next-group scans ----
            SEGS = [(0, 4), (4, 8), (8, 11), (11, 14)]   # lhsT pair ranges
            pend = {}
            for g_ in range(NG):
                for i in range(G):
                    if g_ + 1 < NG and 3 <= i <= 6:
                        si = i - 3
                        if si == 0:
                            pend[g_ + 1] = scan_mm_pairs(g_ + 1)
                        a, b = SEGS[si]
                        for kk in range(a, b):
                            pend[g_ + 1][2](kk)
                        if si == 3:
                            scan_finish(g_ + 1, pend[g_ + 1][0],
                                        pend[g_ + 1][1])
                    pass2_chunk(g_, i)
    nc.compile()
    return nc


def _prep_inputs(x, alpha, beta, gamma):
    import ml_dtypes
    bf = ml_dtypes.bfloat16
    xs = np.asarray(x, dtype=np.float32).reshape(B, L)
    wp, wq, ws1 = _build_coeffs(float(alpha), float(beta), float(gamma))
    wall0 = np.zeros((KS, WALLW), np.float32)
    wall0[:, WP0:WP0 + 3 * C] = wp
    wall0[0:C, WQ0:WQ0 + G * 126] = wq
    wall0[0:9, WS0:WS0 + 126] = ws1
    in_maps = []
    for m in range(NCORES):
        xm = xs[m * BL:(m + 1) * BL]
        xT = np.ascontiguousarray(xm.T)
        xb = xT.astype(bf)
        xin = np.ascontiguousarray(
            xb[1:L].reshape(NCH, C, BL).transpose(1, 0, 2)).reshape(
                C, NCH * BL)
        s0 = np.zeros((9, BL), np.float32)
        s0[0] = xT[0]
        s0[1] = xT[1] - xT[0]
        for j in range(1, P):
            s0[2 + j] = xT[j] - xT[0]
        s0h = s0.astype(bf)
        s0l = (s0 - s0h.astype(np.float32)).astype(bf)
        wall = wall0.copy()
        wall[0:9, S00:S00 + BL] = s0h
        wall[0:9, S00 + BL:S00 + 2 * BL] = s0l
        in_maps.append({"xin": xin, "wall": wall.astype(bf)})
    return in_maps


LAST_RESULT = None

def _ensure_ntff_hook():
    """If BASS_TRACE is set but this environment lacks antenv.axon_hooks
    (concourse imports it under axon when tracing), provide it -- registered
    from the injected libaxon_pjrt.so when available, else a no-op so
    run_bass_kernel_spmd degrades to an untraced run instead of crashing."""
    import importlib.util
    try:
        if importlib.util.find_spec("antenv.axon_hooks") is not None:
            return
    except (ImportError, ModuleNotFoundError, ValueError):
        pass
    import contextlib
    import ctypes
    import sys
    import types

    mod = types.ModuleType("antenv.axon_hooks")
    mod._hook = None
    mod.set_axon_ntff_profile_hook = lambda h: setattr(mod, "_hook", h)
    mod.get_axon_ntff_profile_hook = lambda: mod._hook
    sys.modules["antenv.axon_hooks"] = mod
    try:
        import antenv
        antenv.axon_hooks = mod
    except ImportError:
        pass
    try:
        lib = ctypes.CDLL("/opt/axon/libaxon_pjrt.so")
        if not hasattr(lib, "axon_start_nrt_profile"):
            return
        lib.axon_start_nrt_profile.argtypes = [
            ctypes.POINTER(ctypes.c_int64), ctypes.c_size_t]
        lib.axon_start_nrt_profile.restype = ctypes.c_int64
        lib.axon_stop_nrt_profile.argtypes = [ctypes.c_char_p]
        lib.axon_stop_nrt_profile.restype = ctypes.c_int64

        @contextlib.contextmanager
        def _hook(output_dir, device_ids):
            import jax
            jax.devices()
            if device_ids:
                ids = (ctypes.c_int64 * len(device_ids))(*device_ids)
                rc = lib.axon_start_nrt_profile(ids, len(device_ids))
            else:
                rc = lib.axon_start_nrt_profile(None, 0)
            if rc != 0:
                raise RuntimeError(f"axon_start_nrt_profile rc={rc}")
            try:
                yield
            finally:
                lib.axon_stop_nrt_profile(str(output_dir).encode())

        mod.set_axon_ntff_profile_hook(_hook)
    except OSError:
        pass



def kernel(x, alpha, beta, gamma):
    global LAST_RESULT
    _ensure_ntff_hook()
    from concourse.bass_utils import run_bass_kernel_spmd

    nc = build_bass(BL)
    in_maps = _prep_inputs(x, alpha, beta, gamma)
    res = run_bass_kernel_spmd(nc, in_maps, core_ids=list(range(NCORES)))
    LAST_RESULT = res
    xs = np.asarray(x, dtype=np.float32).reshape(B, L)
    y = np.empty((B, L, 3), np.float32)
    y[:, 0, 0] = xs[:, 0]
    y[:, 0, 1] = xs[:, 1] - xs[:, 0]
    y[:, 0, 2] = 0.0
    for m in range(NCORES):
        o = res.results[m]["out"]
        # ot layout per chunk: ch-major [c0h0|c0h1|c1h0|c1h1|c2h0|c2h1]
        o = o.reshape(C, NCH, 3, 2, NH).astype(np.float32)
        y[m * BL:(m + 1) * BL, 1:, :] = o.transpose(3, 4, 1, 0, 2).reshape(
            BL, L - 1, 3)
    return y


# revision 5
# speedup vs baseline: 1.1098x; 1.1098x over previous
"""Holt-Winters exponential smoothing (level/trend/seasonal, P=7) on 8 Trainium2
NeuronCores — v13: single-ring DMA (SWDGE) + warm PE.

v12's trace exposed the decisive pathology: HWDGE rings (sync/scalar) run
at ~20 GB/s while the SWDGE ring has bulk work queued, and their completion
sems lag 20-30 us (s0's sem hit 16 only at 33 us; the whole scan waited).
v13 therefore routes EVERY transfer through the one ring that sustains line
rate -- gpsimd/SWDGE -- in dependency-safe program order:
  [s0, wall, xg0 pieces, xg1, xg2, g0 scatters, stores (scatters for g1/g2
  interleaved two-per-chunk after the segmented scans)].
Kept from v12: 48-matmul PE warm-up (HAM 8/8 before the scan), pairwise
h0/h1 scan chains, segmented g1/g2 scans between pass-2 chunks i=3..6,
3 x (105,1024) two-bank PSUM tiles with wide alternating DVE/ACT casts,
ch-major ot layout. Math identical to v11 (relL2 ~4.5e-3).
"""

import numpy as np

P = 7
C = 105
G = 13
NG = 3
NCH = G * NG
KS = 114          # pass-2 rhs rows: 105 X + 9 sigma_hi
L = 4096
B = 8192
NCORES = 8
BL = B // NCORES
NH = 512

# wall (weights+s0) column offsets
WP0 = 0           # (114, 315)
WQ0 = 315         # (105, 1638)
WS0 = WP0 + 315 + 1638          # ws1 (9, 126)
S00 = WS0 + 126                 # s0 (9, 2048)
WALLW = S00 + 2 * BL            # 4127

NDUM = 48         # PE warm-up matmuls (N=128, ~4.3us cold -> HAM 8/8)


def _sigmoid(z):
    return 1.0 / (1.0 + np.exp(-z))


def _step_mats(a, b, g):
    A, c = [], []
    for i in range(P):
        col = 2 + i
        Ai = np.zeros((9, 9), np.float64)
        ci = np.zeros(9, np.float64)
        Ai[0, 0] = 1 - a
        Ai[0, 1] = 1 - a
        Ai[0, col] += -a
        Ai[1, 0] = -a * b
        Ai[1, 1] = 1 - a * b
        Ai[1, col] += -a * b
        for j in range(P):
            Ai[2 + j, 2 + j] = 1.0
        Ai[col, :] = 0.0
        Ai[col, 0] = -g * (1 - a)
        Ai[col, 1] = -g * (1 - a)
        Ai[col, col] = g * a + 1 - g
        ci[0] = a
        ci[1] = a * b
        ci[col] = g * (1 - a)
        A.append(Ai)
        c.append(ci)
    return A, c


def _build_coeffs(alpha, beta, gamma):
    """Weight blocks in float64; packed into the per-core wall later."""
    a, b, g = _sigmoid(alpha), _sigmoid(beta), _sigmoid(gamma)
    A, c = _step_mats(a, b, g)
    slots = [(1 + k) % P for k in range(C)]

    Phi = np.zeros((C, 9, 9), np.float64)
    w = np.zeros((C, C, 9), np.float64)
    cur = np.eye(9)
    for k in range(C):
        i = slots[k]
        if k > 0:
            w[k, :k] = w[k - 1, :k] @ A[i].T
        w[k, k] = c[i]
        cur = A[i] @ cur
        Phi[k] = cur
    T = Phi[C - 1]
    V = w[C - 1].T.copy()

    wp = np.zeros((KS, 3 * C), np.float64)          # [ch0|ch1|ch2]
    for k in range(C):
        sel = [0, 1, 2 + slots[k]]
        for ch in range(3):
            wp[105:114, ch * C + k] = Phi[k][sel[ch]]
            for j in range(k + 1):
                wp[j, ch * C + k] = w[k, j][sel[ch]]

    Tpow = [np.eye(9)]
    for _ in range(G + 1):
        Tpow.append(T @ Tpow[-1])

    ws1 = np.zeros((9, 126), np.float64)
    ws1[:, 0:9] = Tpow[G].T
    for j in range(G):
        ws1[:, 9 + 9 * j:18 + 9 * j] = Tpow[j].T
    wq = np.zeros((C, G * 126), np.float64)         # [i0|i1|...|i12]
    for i in range(G):
        blk = wq[:, i * 126:(i + 1) * 126]
        blk[:, 0:9] = (Tpow[G - 1 - i] @ V).T
        for j in range(i + 1, G):
            blk[:, 9 + 9 * j:18 + 9 * j] = (Tpow[j - 1 - i] @ V).T

    return wp, wq, ws1


def build_bass(bl=BL):
    import concourse.bacc as bacc
    import concourse.mybir as mybir
    from concourse.tile import TileContext

    BF = mybir.dt.bfloat16
    F32 = mybir.dt.float32
    COPY = mybir.ActivationFunctionType.Copy
    GW = G * bl

    nc = bacc.Bacc(None, target_bir_lowering=False, debug=False)
    xin = nc.declare_dram_parameter("xin", [C, NCH * bl], BF, isOutput=False)
    wall_d = nc.declare_dram_parameter("wall", [KS, WALLW], BF,
                                       isOutput=False)
    out_d = nc.declare_dram_parameter("out", [C, NCH * 3 * bl], BF,
                                      isOutput=True)

    with TileContext(nc) as tc:
        with (
            tc.tile_pool(name="consts", bufs=1) as consts,
            tc.tile_pool(name="xpool", bufs=NG) as xpool,
            tc.tile_pool(name="spool", bufs=2) as spool,
            tc.tile_pool(name="ypool", bufs=8) as ypool,
            tc.tile_pool(name="ypsum", bufs=3, space="PSUM") as ypsum,
            tc.tile_pool(name="spsum", bufs=2, space="PSUM") as spsum,
        ):
            cw = consts.tile([KS, WALLW], BF)
            # s0 block first (tiny, gates the first scan matmuls), then the
            # weight block; rows 9:114 of the s0 region are never read.
            # Everything rides the SWDGE ring (see module docstring).
            nc.gpsimd.dma_start(out=cw[0:9, S00:WALLW], in_=wall_d[0:9, S00:WALLW])
            nc.gpsimd.dma_start(out=cw[:, 0:S00], in_=wall_d[:, 0:S00])
            wp = cw[:, WP0:WP0 + 3 * C]
            wq = cw[0:C, WQ0:WQ0 + G * 126]
            ws1 = cw[0:9, WS0:WS0 + 126]
            s0 = cw[0:9, S00:S00 + 2 * bl]

            # PE warm-up scratch (memset on gpsimd, runs in the preamble)
            dum = consts.tile([128, 128], BF)
            nc.gpsimd.memset(dum[:], 0.0)

            # xg0 in 2-chunk pieces, all on the SWDGE ring ahead of xg1/xg2
            # so the scan is never input-starved.
            xg = []
            xt0 = xpool.tile([KS, GW], BF, tag="xg", name="xg0")
            for (a, b) in [(0, 2), (2, 4), (4, 6), (6, 8),
                           (8, 10), (10, 12), (12, 13)]:
                nc.gpsimd.dma_start(out=xt0[0:C, a * bl:b * bl],
                                    in_=xin[:, a * bl:b * bl])
            xg.append(xt0)

            # PE warm-up: back-to-back N=128 matmuls into a scratch PSUM
            # tile (spsum buf 0, recycled before the scan needs it). ~4.3us
            # of continuous PE busy flips HAM to 8/8 before the scan.
            dps = spsum.tile([126, NH], F32, tag="sp", name="dps")
            for _ in range(NDUM):
                nc.tensor.matmul(dps[0:126, 0:128], lhsT=dum[:, 0:126],
                                 rhs=dum[:, 0:128], start=True, stop=True)

            # xg1 then xg2, behind xg0 on the same ring.
            xt1 = xpool.tile([KS, GW], BF, tag="xg", name="xg1")
            for (a, b) in [(0, 7), (7, 13)]:
                nc.gpsimd.dma_start(out=xt1[0:C, a * bl:b * bl],
                                    in_=xin[:, GW + a * bl:GW + b * bl])
            xg.append(xt1)
            xt2 = xpool.tile([KS, GW], BF, tag="xg", name="xg2")
            for (a, b) in [(0, 7), (7, 13)]:
                nc.gpsimd.dma_start(out=xt2[0:C, a * bl:b * bl],
                                    in_=xin[:, 2 * GW + a * bl:2 * GW + b * bl])
            xg.append(xt2)

            state = [s0[:, 0:bl]]
            sg_tiles = []

            def scan_mm_pairs(g_):
                """The 14 (lhsT, rhs-col) matmul pairs of group g_'s scan,
                h0/h1 chains interleaved so each lhsT is used twice in a row
                (LDW amortized by the PE reorder window)."""
                st = state[g_]
                sp0 = spsum.tile([126, NH], F32, tag="sp", name=f"sp{g_}_0")
                sp1 = spsum.tile([126, NH], F32, tag="sp", name=f"sp{g_}_1")
                pairs = []

                def emit(k):
                    if k == 0:
                        lh, r0, r1 = ws1, st[:, 0:NH], st[:, NH:2 * NH]
                        nc.tensor.matmul(sp0[:], lhsT=lh, rhs=r0,
                                         start=True, stop=False)
                        nc.tensor.matmul(sp1[:], lhsT=lh, rhs=r1,
                                         start=True, stop=False)
                    else:
                        i = k - 1
                        lh = wq[:, i * 126:(i + 1) * 126]
                        base = i * bl
                        last = (i == G - 1)
                        nc.tensor.matmul(
                            sp0[:], lhsT=lh,
                            rhs=xg[g_][0:C, base:base + NH],
                            start=False, stop=last)
                        nc.tensor.matmul(
                            sp1[:], lhsT=lh,
                            rhs=xg[g_][0:C, base + NH:base + 2 * NH],
                            start=False, stop=last)
                return sp0, sp1, emit

            def scan_finish(g_, sp0, sp1):
                """Casts after the chains complete; scatters issued by the
                caller at ring-friendly points."""
                sg = spool.tile([126, bl], BF, tag="sg", name=f"sg{g_}")
                nc.scalar.activation(out=sg[:, 0:NH], in_=sp0[:], func=COPY)
                nc.scalar.activation(out=sg[:, NH:2 * NH], in_=sp1[:],
                                     func=COPY)
                state.append(sg[0:9, :])
                sg_tiles.append(sg)

            def scatter(g_, i):
                sg = sg_tiles[g_]
                nc.gpsimd.dma_start(
                    out=xg[g_][105:114, i * bl:(i + 1) * bl],
                    in_=sg[9 + 9 * i:18 + 9 * i, :])

            def pass2_chunk(g_, i):
                k = g_ * G + i
                ot = ypool.tile([C, 3 * bl], BF, tag="ot", name=f"ot{k}")
                for ch in range(3):
                    yp = ypsum.tile([C, bl], F32, tag="yp",
                                    name=f"yp{k}_{ch}")
                    for h in range(2):
                        nc.tensor.matmul(
                            yp[:, h * NH:(h + 1) * NH],
                            lhsT=wp[:, ch * C:(ch + 1) * C],
                            rhs=xg[g_][0:KS, i * bl + h * NH:
                                       i * bl + (h + 1) * NH],
                            start=True, stop=True)
                    oc = slice(ch * bl, (ch + 1) * bl)
                    if (k * 3 + ch) % 2 == 0:
                        nc.vector.tensor_copy(out=ot[:, oc], in_=yp[:])
                    else:
                        nc.scalar.activation(out=ot[:, oc], in_=yp[:],
                                             func=COPY)
                c0 = k * 3 * bl
                nc.gpsimd.dma_start(out=out_d[:, c0:c0 + 3 * bl], in_=ot[:])

            # ---- group 0 scan (monolithic: competes only with dummies) ----
            sp0, sp1, emit = scan_mm_pairs(0)
            for kk in range(1 + G):
                emit(kk)
            scan_finish(0, sp0, sp1)
            for i in range(G):
                scatter(0, i)

            # ---- pass-2 with segmented next-group scans ----
            SEGS = [(0, 4), (4, 8), (8, 11), (11, 14)]   # lhsT pair ranges
            pend = {}
            for g_ in range(NG):
                for i in range(G):
                    if g_ + 1 < NG and 3 <= i <= 6:
                        si = i - 3
                        if si == 0:
                            pend[g_ + 1] = scan_mm_pairs(g_ + 1)
                        a, b = SEGS[si]
                        for kk in range(a, b):
                            pend[g_ + 1][2](kk)
                        if si == 3:
                            scan_finish(g_ + 1, pend[g_ + 1][0],
                                        pend[g_ + 1][1])
                    pass2_chunk(g_, i)
                    # next group's sigma scatters, two per chunk slot from
                    # i=6 on: ring-ordered after this chunk's store, ready
                    # well before pass-2 of group g_+1 reaches them.
                    if g_ + 1 < NG and i >= 6:
                        for j in (2 * (i - 6), 2 * (i - 6) + 1):
                            if j < G:
                                scatter(g_ + 1, j)
    nc.compile()
    return nc


def _prep_inputs(x, alpha, beta, gamma):
    import ml_dtypes
    bf = ml_dtypes.bfloat16
    xs = np.asarray(x, dtype=np.float32).reshape(B, L)
    wp, wq, ws1 = _build_coeffs(float(alpha), float(beta), float(gamma))
    wall0 = np.zeros((KS, WALLW), np.float32)
    wall0[:, WP0:WP0 + 3 * C] = wp
    wall0[0:C, WQ0:WQ0 + G * 126] = wq
    wall0[0:9, WS0:WS0 + 126] = ws1
    in_maps = []
    for m in range(NCORES):
        xm = xs[m * BL:(m + 1) * BL]
        xT = np.ascontiguousarray(xm.T)
        xb = xT.astype(bf)
        xin = np.ascontiguousarray(
            xb[1:L].reshape(NCH, C, BL).transpose(1, 0, 2)).reshape(
                C, NCH * BL)
        s0 = np.zeros((9, BL), np.float32)
        s0[0] = xT[0]
        s0[1] = xT[1] - xT[0]
        for j in range(1, P):
            s0[2 + j] = xT[j] - xT[0]
        s0h = s0.astype(bf)
        s0l = (s0 - s0h.astype(np.float32)).astype(bf)
        wall = wall0.copy()
        wall[0:9, S00:S00 + BL] = s0h
        wall[0:9, S00 + BL:S00 + 2 * BL] = s0l
        in_maps.append({"xin": xin, "wall": wall.astype(bf)})
    return in_maps


LAST_RESULT = None

def _ensure_ntff_hook():
    """If BASS_TRACE is set but this environment lacks antenv.axon_hooks
    (concourse imports it under axon when tracing), provide it -- registered
    from the injected libaxon_pjrt.so when available, else a no-op so
    run_bass_kernel_spmd degrades to an untraced run instead of crashing."""
    import importlib.util
    try:
        if importlib.util.find_spec("antenv.axon_hooks") is not None:
            return
    except (ImportError, ModuleNotFoundError, ValueError):
        pass
    import contextlib
    import ctypes
    import sys
    import types

    mod = types.ModuleType("antenv.axon_hooks")
    mod._hook = None
    mod.set_axon_ntff_profile_hook = lambda h: setattr(mod, "_hook", h)
    mod.get_axon_ntff_profile_hook = lambda: mod._hook
    sys.modules["antenv.axon_hooks"] = mod
    try:
        import antenv
        antenv.axon_hooks = mod
    except ImportError:
        pass
    try:
        lib = ctypes.CDLL("/opt/axon/libaxon_pjrt.so")
        if not hasattr(lib, "axon_start_nrt_profile"):
            return
        lib.axon_start_nrt_profile.argtypes = [
            ctypes.POINTER(ctypes.c_int64), ctypes.c_size_t]
        lib.axon_start_nrt_profile.restype = ctypes.c_int64
        lib.axon_stop_nrt_profile.argtypes = [ctypes.c_char_p]
        lib.axon_stop_nrt_profile.restype = ctypes.c_int64

        @contextlib.contextmanager
        def _hook(output_dir, device_ids):
            import jax
            jax.devices()
            if device_ids:
                ids = (ctypes.c_int64 * len(device_ids))(*device_ids)
                rc = lib.axon_start_nrt_profile(ids, len(device_ids))
            else:
                rc = lib.axon_start_nrt_profile(None, 0)
            if rc != 0:
                raise RuntimeError(f"axon_start_nrt_profile rc={rc}")
            try:
                yield
            finally:
                lib.axon_stop_nrt_profile(str(output_dir).encode())

        mod.set_axon_ntff_profile_hook(_hook)
    except OSError:
        pass



def kernel(x, alpha, beta, gamma):
    global LAST_RESULT
    _ensure_ntff_hook()
    from concourse.bass_utils import run_bass_kernel_spmd

    nc = build_bass(BL)
    in_maps = _prep_inputs(x, alpha, beta, gamma)
    res = run_bass_kernel_spmd(nc, in_maps, core_ids=list(range(NCORES)))
    LAST_RESULT = res
    xs = np.asarray(x, dtype=np.float32).reshape(B, L)
    y = np.empty((B, L, 3), np.float32)
    y[:, 0, 0] = xs[:, 0]
    y[:, 0, 1] = xs[:, 1] - xs[:, 0]
    y[:, 0, 2] = 0.0
    for m in range(NCORES):
        o = res.results[m]["out"]
        # ot layout per chunk: ch-major [c0h0|c0h1|c1h0|c1h1|c2h0|c2h1]
        o = o.reshape(C, NCH, 3, 2, NH).astype(np.float32)
        y[m * BL:(m + 1) * BL, 1:, :] = o.transpose(3, 4, 1, 0, 2).reshape(
            BL, L - 1, 3)
    return y
